# revision 45
# baseline (speedup 1.0000x reference)
"""Trainium2 Bass kernel for nn_Attention (B=4, T=1024, C=1024, 16 heads).

Sharding v2: 8 cores = (batch b, head-half hh). Core i = (b=i//2, hh=i%2)
computes heads [hh*8, hh*8+8) of batch b for ALL 1024 queries, plus the
partial output projection over its 512 channels. The host sums the two
partial y's per batch during unsharding (a host-side reduce instead of
an on-device collective; assembly was already host-side in v1).

vs v1 ((batch, query-half) sharding, 195.7us): K and V were projected
for the full batch on both cores of a pair (33% redundant FLOPs). Here
every projection is done exactly once: per-core matmul slots drop from
584 to 456 (~29us of TensorE stream), and weight DMA halves (12MB vs
20MB fp32), shrinking the DMA-paced head.

On-chip layout identical to v1 (no transposes anywhere):
  - host passes x^T; K^T/Q^T come out of their projections in [o, t]
  - V in [t, o] with a leading ones-column per head so the PV matmul's
    PSUM row 0 is the softmax denominator Z for free
  - softmax on S^T (keys on partitions) with no max-subtraction
  - normalization: custom-DVE reciprocal of the Z row, GpSimd
    partition_broadcast, one DVE multiply
  - output projection consumes out^T directly, yields y^T (+bias on
    ScalarE, applied only on hh=0 cores; host sums partials)

S^T matmuls contract over head_dim=64 at base partitions 0/64 -> bass
auto-emits them as 64x128 row-tiles T0/T8 which the PE runs
concurrently (verified in the v1 trace: ~4ns start gaps).

Schedule: groups are (pair p, q-half qh), pair-major. q-half-0 groups
are thin (S+PV only); the next pair's K^T/Q^T projections are emitted
inside the q-half-1 groups so every group has >= ~12us of TensorE work
to hide its ~8.8us of ScalarE exp. V streams behind the wv DMA right
after the first S unit; Wproj prefetches ride the idle DMA mid-kernel.
A short keep-warm matmul chain bridges the final norm drain into the
output projection.

All matmuls float32r (TF32-like) with fp32 PSUM accumulation;
KERNEL_BF16=1 switches inputs to bf16 (halves DMA bytes, same matmul
rate, ~4e-3 rel err instead of ~4e-4).
"""

import numpy as np

B, T, C = 4, 1024, 1024
NH, HD = 16, 64
NHL = NH // 2          # heads per core
TQ = T // 2            # q-half tile
KC = C // 128          # 8 contraction chunks over C (x channels)
OC = NHL * HD // 128   # 4 output chunks for K/Q (= local qkv channels/128)
SCALE = 1.0 / float(np.sqrt(HD))

_PROG = None
import os
USE_BF16 = os.environ.get("KERNEL_BF16", "0") == "1"


def _build_program():
    import concourse.bacc as bacc
    import concourse.bass_isa as bass_isa
    import concourse.mybir as mybir
    import concourse.tile as tile

    F32 = mybir.dt.float32
    F32R = mybir.dt.bfloat16 if USE_BF16 else mybir.dt.float32r
    BF16 = mybir.dt.bfloat16
    Exp = mybir.ActivationFunctionType.Exp
    Ident = mybir.ActivationFunctionType.Identity

    def r(ap):
        return ap.bitcast(F32R)

    nc = bacc.Bacc()
    xt_d = nc.declare_dram_parameter("xt", [KC, 128, T], F32R, isOutput=False)
    wq_d = nc.declare_dram_parameter("wq", [OC, 128, C], F32R, isOutput=False)
    wk_d = nc.declare_dram_parameter("wk", [OC, 128, C], F32R, isOutput=False)
    wv_d = nc.declare_dram_parameter("wv", [KC, 128, NHL * 128 // 2], F32R,
                                     isOutput=False)
    wp_d = nc.declare_dram_parameter("wp", [OC, 128, C], F32R, isOutput=False)
    bias_d = nc.declare_dram_parameter("bias", [128, 8], F32, isOutput=False)
    ones_d = nc.declare_dram_parameter("ones", [128, 128], F32R, isOutput=False)
    yt_d = nc.declare_dram_parameter("yt", [8, 128, T], F32, isOutput=True)

    from contextlib import ExitStack

    with ExitStack() as ctx:
        tc = ctx.enter_context(tile.TileContext(nc))
        ctx.enter_context(
            nc.allow_low_precision(
                "float32r matmul inputs (TF32-like) are intentional"
            )
        )
        pool = lambda name, bufs, **kw: ctx.enter_context(  # noqa: E731
            tc.tile_pool(name=name, bufs=bufs, **kw)
        )
        xt_pool = pool("xt", KC)
        wstat_pool = pool("wstat", 3)
        wv_pool = pool("wv", KC)
        wp_pool = pool("wp", OC)
        kt_pool = pool("kt", 2)
        qt_pool = pool("qt", 2)
        v_pool = pool("vaug", KC)
        exp_pool = pool("exp", 10)
        ot_pool = pool("ot", OC)
        y_pool = pool("ysb", 4)
        osb2_pool = pool("osb2", 2)
        r_pool = pool("rsb", 2)
        rbx_pool = pool("rbx", 2)
        bias_pool = pool("bias", 8)
        ps_proj = pool("psproj", 2, space="PSUM")
        ps_s = pool("pss", 2, space="PSUM")  # [128,1024] pair tiles, 2 banks
        ps_ops = pool("psops", 2, space="PSUM")

        # ---- stage 0: streamed loads ----------------------------------
        # The first-half weights go out on the ScalarE HWDGE ring (its
        # preamble finishes earlier than Sync's and the rings dispatch
        # in parallel); xt and everything else stream on Sync. Only
        # 0.83MB precedes xt0, and the n=1 weight halves (contraction
        # chunks 4-7, not needed until xt4) follow xt0.
        ones_sb = bias_pool.tile([128, 128], F32R, tag="ones", name="ones_sb")
        nc.scalar.dma_start(ones_sb[:], ones_d[:])

        def rf(ap):
            # view an F32R-declared tile as plain fp32 for DVE arithmetic
            return ap if USE_BF16 else ap.bitcast(F32)
        kw0 = wstat_pool.tile([128, C], F32R, tag="wstat", name="kw0")
        nc.scalar.dma_start(kw0[:, 0:TQ], wk_d[0][:, 0:TQ])
        qw0 = wstat_pool.tile([128, C], F32R, tag="wstat", name="qw0")
        nc.scalar.dma_start(qw0[:, 0:TQ], wq_d[0][:, 0:TQ])
        xt = []
        for k in range(KC):
            t_ = xt_pool.tile([128, T], F32R, tag="xt", name=f"xt{k}")
            nc.sync.dma_start(t_[:], xt_d[k])
            xt.append(t_)
            if k == 0:
                nc.sync.dma_start(kw0[:, TQ:C], wk_d[0][:, TQ:C])
                nc.sync.dma_start(qw0[:, TQ:C], wq_d[0][:, TQ:C])

        VW = NHL * 64  # 512 V output cols
        wv_sb = []
        for k in range(KC):
            wvt = wv_pool.tile([128, VW], F32R, tag="wv", name=f"wv{k}")
            nc.sync.dma_start(wvt[:], wv_d[k])
            wv_sb.append(wvt)
        va = []
        for m in range(KC):
            vt = v_pool.tile([128, NHL * 65], F32R, tag="vaug", name=f"va{m}")
            view = vt[:].rearrange("p (h e) -> p h e", e=65)
            # ones column FIRST per head: the PV output's Z row lands on
            # partition 0 (softmax denominator for free).
            nc.vector.tensor_copy(view[:, :, 0:1], ones_sb[:, 0:NHL].unsqueeze(2))
            va.append(vt)

        bias_t = bias_pool.tile([128, 8], F32, tag="bias", name="bias_t")
        nc.sync.dma_start(bias_t[:], bias_d[:])
        bias_sb = [bias_t[:, m:m + 1] for m in range(8)]

        ot = []
        for k in range(OC):
            o_ = ot_pool.tile([128, T], F32R, tag="ot", name=f"ot{k}")
            ot.append(o_)

        def emit_proj_half(t_, w, p, n, which):
            # one T/2 half of a K^T or Q^T chunk: [128 (2 heads x 64d), TQ]
            ps = ps_proj.tile([128, TQ], F32, tag="ps", name=f"{which}{p}{n}")
            for k in range(KC):
                nc.tensor.matmul(
                    ps[:], r(w[:, k * 128:(k + 1) * 128]),
                    r(xt[k][:, n * TQ:(n + 1) * TQ]),
                    start=(k == 0), stop=(k == KC - 1),
                )
            nc.vector.tensor_copy(t_[:, n * TQ:(n + 1) * TQ], ps[:])

        def emit_proj(p, w, which):
            t_ = (kt_pool if which == "k" else qt_pool).tile(
                [128, T], F32R,
                tag="kt" if which == "k" else "qt", name=f"{which}t{p}")
            for n in range(2):
                emit_proj_half(t_, w, p, n, which)
            return t_

        def emit_s_pair(p, qh, k_, q_):
            # Both heads' S^T chunk j share one 2-bank PSUM tile so a
            # single [128,1024] exp covers them. The two matmuls are
            # 64x128 row-tiles (base partitions 0/64) -> concurrent.
            exps = []
            for j in range(KC):
                sps = ps_s.tile([128, 2 * TQ], F32, tag="ps", name=f"s{p}{qh}{j}")
                nc.tensor.matmul(
                    sps[:, 0:TQ],
                    r(k_[0:64, j * 128:(j + 1) * 128]),
                    r(q_[0:64, qh * TQ:(qh + 1) * TQ]),
                    start=True, stop=True,
                )
                nc.tensor.matmul(
                    sps[:, TQ:2 * TQ],
                    r(k_[64:128, j * 128:(j + 1) * 128]),
                    r(q_[64:128, qh * TQ:(qh + 1) * TQ]),
                    start=True, stop=True,
                )
                e = exp_pool.tile([128, 2 * TQ], F32R, tag="exp",
                                  name=f"e{p}{qh}{j}")
                nc.scalar.activation(e[:], sps[:], Exp, scale=SCALE)
                exps.append(e)
            return exps

        def emit_norm(hl, qh, ops, fast=False):
            okc, half = divmod(hl, 2)
            po = half * 64
            if fast:
                # Last-unit path: minimum-latency chain straight from
                # PSUM (nothing waits on the ps_ops buf anymore).
                src = ops[0:65, :]
            else:
                # Evacuate PSUM first: a single DVE copy frees the
                # ps_ops buf for the next PV ~2.3us earlier than holding
                # it through the whole recip/broadcast/mul chain (which
                # paced PV starts in the 2-buf rotation).
                src = osb2_pool.tile([65, TQ], F32, tag="oraw",
                                     name=f"or{hl}{qh}")[:]
                nc.vector.tensor_copy(src, ops[0:65, :])
            rt0 = r_pool.tile([1, TQ], F32, tag="rsb", name=f"r0{hl}{qh}")
            nc.vector.reciprocal_approx_fast(rt0[0:1, :], src[0:1, :])
            rbx = rbx_pool.tile([65, TQ], F32, tag="rbx", name=f"rbx{hl}{qh}")
            nc.gpsimd.partition_broadcast(rbx[:], rt0[0:1, :])
            ob2 = osb2_pool.tile([65, TQ], F32R, tag="osb2", name=f"ob2_{hl}{qh}")
            nc.vector.tensor_mul(ob2[:], src, rbx[:])
            nc.sync.dma_start(ot[okc][po:po + 64, qh * TQ:(qh + 1) * TQ],
                              ob2[1:65, :])

        def emit_pv(hl, qh, exps, fast_norm=False):
            lo = (hl % 2) * TQ
            ops = ps_ops.tile([65, TQ], F32, tag="ps", name=f"o{hl}{qh}")
            for j in range(KC):
                nc.tensor.matmul(
                    ops[:], r(va[j][:, hl * 65:(hl + 1) * 65]),
                    r(exps[j][:, lo:lo + TQ]),
                    start=(j == 0), stop=(j == KC - 1),
                )
            emit_norm(hl, qh, ops, fast=fast_norm)

        # ---- pair 0 header: 4-way interleaved chase -------------------
        # All four kt0/qt0 half-accumulators advance together as each xt
        # chunk's DMA lands (4 matmuls per arrival instead of serial
        # half-by-half chasing), with short ones x ones filler matmuls
        # between chunks to keep the PE's HAM activity window warm while
        # the stream is DMA-paced.
        kt = kt_pool.tile([128, T], F32R, tag="kt", name="kt0")
        qt = qt_pool.tile([128, T], F32R, tag="qt", name="qt0")
        ps_k0 = ps_proj.tile([128, TQ], F32, tag="ps", name="ps_k0")
        ps_k1 = ps_proj.tile([128, TQ], F32, tag="ps", name="ps_k1")
        ps_q = ps_s.tile([128, 2 * TQ], F32, tag="ps", name="ps_q01")
        warm0 = ps_ops.tile([128, 128], F32, tag="ps", name="warm0")
        for k in range(KC):
            fl = dict(start=(k == 0), stop=(k == KC - 1))
            nc.tensor.matmul(ps_k0[:], r(kw0[:, k * 128:(k + 1) * 128]),
                             r(xt[k][:, 0:TQ]), **fl)
            nc.tensor.matmul(ps_k1[:], r(kw0[:, k * 128:(k + 1) * 128]),
                             r(xt[k][:, TQ:T]), **fl)
            nc.tensor.matmul(ps_q[:, 0:TQ], r(qw0[:, k * 128:(k + 1) * 128]),
                             r(xt[k][:, 0:TQ]), **fl)
            nc.tensor.matmul(ps_q[:, TQ:T], r(qw0[:, k * 128:(k + 1) * 128]),
                             r(xt[k][:, TQ:T]), **fl)
            if k < KC - 1:
                # fillers read xt[k] so the scheduler can't hoist them
                # ahead of the chase — they run in the DMA-wait window
                # after chunk k's projections, keeping HAM activity up.
                for w in range(2):
                    nc.tensor.matmul(warm0[:, 0:128], r(ones_sb[:]),
                                     r(xt[k][:, 0:128]), start=True, stop=True)
        nc.vector.tensor_copy(kt[:, 0:TQ], ps_k0[:])
        nc.vector.tensor_copy(kt[:, TQ:T], ps_k1[:])
        nc.vector.tensor_copy(qt[:, 0:TQ], ps_q[:, 0:TQ])
        nc.vector.tensor_copy(qt[:, TQ:T], ps_q[:, TQ:T])
        exps = emit_s_pair(0, 0, kt, qt)

        # ---- V = x @ Wv^T ([t,o] + ones cols), streams behind DMA -----
        for m in range(KC):
            view = va[m][:].rearrange("p (h e) -> p h e", e=65)
            ps = ps_proj.tile([128, VW], F32, tag="ps", name=f"v{m}")
            for k in range(KC):
                nc.tensor.matmul(
                    ps[:], r(xt[k][:, m * 128:(m + 1) * 128]),
                    r(wv_sb[k][:]),
                    start=(k == 0), stop=(k == KC - 1),
                )
            src = ps[:].rearrange("p (h d) -> p h d", d=64)
            nc.vector.tensor_copy(view[:, 0:NHL, 1:65], src)

        # ---- groups, pair-major --------------------------------------
        # Each group of pair p also carries HALF of pair p+1's K^T/Q^T
        # projections (16 matmul slots) so every group has ~9us of
        # TensorE work against its ~8.8us of ScalarE exp; Wproj
        # prefetches ride the idle DMA.
        wp_sb = {}

        def prefetch_wp(m):
            yw = wp_pool.tile([128, C], F32R, tag="wp", name=f"yw{m}")
            nc.sync.dma_start(yw[:], wp_d[m])
            wp_sb[m] = yw

        # y^T partial = Wproj_half @ out^T (+ b on hh=0). wp_sb[k]
        # covers contraction chunk k (128 of the core's 512 channels)
        # with all 1024 outputs in its cols; output chunk m uses cols
        # m*128:(m+1)*128. Blocks rotate across ALL three PSUM pools
        # (S/PV pools are dead by then) so the bias-ACT evacuation never
        # gates the matmul stream. th=0 blocks only need the q-half-0
        # norms, so they run inside the last group (filling its exp
        # window); th=1 blocks are the kernel tail.
        y_ps_pools = [ps_proj, ps_s, ps_ops]

        def emit_y_half(th):
            for m in range(8):
                ps = y_ps_pools[m % 3].tile([128, TQ], F32, tag="ps",
                                            name=f"y{m}{th}")
                for k in range(OC):
                    nc.tensor.matmul(
                        ps[:], r(wp_sb[k][:, m * 128:(m + 1) * 128]),
                        r(ot[k][:, th * TQ:(th + 1) * TQ]),
                        start=(k == 0), stop=(k == OC - 1),
                    )
                ysb = y_pool.tile([128, TQ], F32, tag="ysb", name=f"ysb{m}{th}")
                nc.scalar.activation(ysb[:], ps[:], Ident, bias=bias_sb[m])
                nc.sync.dma_start(yt_d[m][:, th * TQ:(th + 1) * TQ], ysb[:])

        kt_nx = qt_nx = kw_nx = qw_nx = None
        for g in range(2 * OC):
            p, qh = divmod(g, 2)
            if g > 0:
                exps = emit_s_pair(p, qh, kt, qt)
            if g == 2 * OC - 1:
                prefetch_wp(OC - 1)
            if qh == 0:
                if p + 1 < OC:
                    kw_nx = wstat_pool.tile([128, C], F32R, tag="wstat",
                                            name=f"kw{p + 1}")
                    nc.sync.dma_start(kw_nx[:], wk_d[p + 1])
                    qw_nx = wstat_pool.tile([128, C], F32R, tag="wstat",
                                            name=f"qw{p + 1}")
                    nc.sync.dma_start(qw_nx[:], wq_d[p + 1])
                    kt_nx = kt_pool.tile([128, T], F32R, tag="kt",
                                         name=f"kt{p + 1}")
                    qt_nx = qt_pool.tile([128, T], F32R, tag="qt",
                                         name=f"qt{p + 1}")
                    emit_proj_half(kt_nx, kw_nx, p + 1, 0, "k")
                    emit_proj_half(qt_nx, qw_nx, p + 1, 0, "q")
            else:
                if p + 1 < OC:
                    emit_proj_half(kt_nx, kw_nx, p + 1, 1, "k")
                    emit_proj_half(qt_nx, qw_nx, p + 1, 1, "q")
            if g == 2 * OC - 1:
                emit_y_half(0)
            last = g == 2 * OC - 1
            emit_pv(2 * p, qh, exps, fast_norm=last)
            emit_pv(2 * p + 1, qh, exps, fast_norm=last)
            if qh == 1:
                if p + 1 < OC:
                    kt, qt = kt_nx, qt_nx
                for m in {1: [0], 2: [1, 2]}.get(p, []):
                    prefetch_wp(m)

        # ---- Y th=1 tail: incremental accumulation --------------------
        # m=0..5 pre-accumulate contraction chunks k=0..2 (their ot
        # chunks were normed a group ago) in ps_proj + ps_s banks while
        # the final norm chains drain (doubles as the HAM keep-warm
        # bridge); only their k=3 stop-matmuls and the m=6,7 blocks
        # (ps_ops, free after the final norm muls) sit past the last
        # norm.
        y_aps = []
        for i in range(2):
            t = ps_proj.tile([128, TQ], F32, tag="ps", name=f"yt1a{i}")
            y_aps.append(t[:])
        for i in range(2):
            t = ps_s.tile([128, 2 * TQ], F32, tag="ps", name=f"yt1c{i}")
            y_aps.append(t[:, 0:TQ])
            y_aps.append(t[:, TQ:2 * TQ])
        for k in range(OC - 1):
            for m in range(6):
                nc.tensor.matmul(
                    y_aps[m], r(wp_sb[k][:, m * 128:(m + 1) * 128]),
                    r(ot[k][:, TQ:T]),
                    start=(k == 0), stop=False,
                    skip_group_check=True,
                )
        for m in range(6):
            nc.tensor.matmul(
                y_aps[m], r(wp_sb[OC - 1][:, m * 128:(m + 1) * 128]),
                r(ot[OC - 1][:, TQ:T]),
                start=False, stop=True,
                skip_group_check=True,
            )
            ysb = y_pool.tile([128, TQ], F32, tag="ysb", name=f"ysb{m}1")
            nc.scalar.activation(ysb[:], y_aps[m], Ident, bias=bias_sb[m])
            nc.sync.dma_start(yt_d[m][:, TQ:T], ysb[:])
        for m in (6, 7):
            ps = ps_ops.tile([128, TQ], F32, tag="ps", name=f"y{m}1")
            for k in range(OC):
                nc.tensor.matmul(
                    ps[:], r(wp_sb[k][:, m * 128:(m + 1) * 128]),
                    r(ot[k][:, TQ:T]),
                    start=(k == 0), stop=(k == OC - 1),
                )
            ysb = y_pool.tile([128, TQ], F32, tag="ysb", name=f"ysb{m}1")
            nc.scalar.activation(ysb[:], ps[:], Ident, bias=bias_sb[m])
            nc.sync.dma_start(yt_d[m][:, TQ:T], ysb[:])

    nc.compile()
    return nc


def _get_program():
    global _PROG
    if _PROG is None:
        _PROG = _build_program()
    return _PROG


def _prep_inputs(x, Wqkv, Wproj, bproj):
    """Host-side shard prep: per-core input maps (contiguous)."""
    x = np.asarray(x, dtype=np.float32)
    Wqkv = np.asarray(Wqkv, dtype=np.float32)
    Wproj = np.asarray(Wproj, dtype=np.float32)
    bproj = np.asarray(bproj, dtype=np.float32)

    mmdt = np.float32
    if USE_BF16:
        import ml_dtypes

        mmdt = ml_dtypes.bfloat16

    def cols(wT):
        # [C, 512] (c, o) -> [OC, 128, C]: per o-chunk column, laid out
        # so one contiguous DMA fills the stationary tile
        # [128p, k*128+d] = wT[k*128+p, oc*128+d]
        return np.ascontiguousarray(
            wT.reshape(KC, 128, OC, 128).transpose(2, 1, 0, 3).reshape(OC, 128, C)
        )

    halves = []
    for hh in range(2):
        sl = slice(hh * 512, (hh + 1) * 512)
        wq = cols(Wqkv[0:C][sl].T).astype(mmdt)
        wk = cols(Wqkv[C:2 * C][sl].T).astype(mmdt)
        wv = np.ascontiguousarray(
            Wqkv[2 * C:3 * C][sl].T.reshape(KC, 128, 512)
        ).astype(mmdt)
        # wp chunk k: [128 c, 1024 o] = Wproj[:, hh*512 + k*128 + p].T
        wp = np.ascontiguousarray(
            Wproj[:, sl].T.reshape(OC, 128, C)
        ).astype(mmdt)
        bias = np.ascontiguousarray(
            (bproj if hh == 0 else np.zeros_like(bproj)).reshape(8, 128).T
        )
        halves.append((wq, wk, wv, wp, bias))
    ones = np.ones((128, 128), dtype=mmdt)

    in_maps = []
    for i in range(8):
        b, hh = divmod(i, 2)
        wq, wk, wv, wp, bias = halves[hh]
        xt = np.ascontiguousarray(x[b].T.reshape(KC, 128, T)).astype(mmdt)
        in_maps.append(
            {
                "xt": xt, "wq": wq, "wk": wk, "wv": wv, "wp": wp,
                "bias": bias, "ones": ones,
            }
        )
    return in_maps


def _assemble(results, x_dtype):
    out = np.empty((B, T, C), dtype=np.float32)
    for b in range(B):
        y0 = results[2 * b]["yt"].reshape(C, T)
        y1 = results[2 * b + 1]["yt"].reshape(C, T)
        out[b] = (y0 + y1).T
    return out.astype(x_dtype, copy=False)


def run(inputs, trace=False, **spmd_kwargs):
    """Shared entry for kernel() and test harnesses (trace for profiling)."""
    from concourse.bass_utils import run_bass_kernel_spmd

    nc = _get_program()
    in_maps = _prep_inputs(**inputs)
    res = run_bass_kernel_spmd(
        nc, in_maps, list(range(8)), trace=trace, **spmd_kwargs
    )
    out = _assemble(res.results, np.asarray(inputs["x"]).dtype)
    return out, res


def kernel(x, Wqkv, Wproj, bproj):
    out, _ = run(dict(x=x, Wqkv=Wqkv, Wproj=Wproj, bproj=bproj))
    return out


# revision 47
# speedup vs baseline: 1.0257x; 1.0257x over previous
"""Trainium2 Bass kernel for nn_Attention (B=4, T=1024, C=1024, 16 heads).

Sharding v2: 8 cores = (batch b, head-half hh). Core i = (b=i//2, hh=i%2)
computes heads [hh*8, hh*8+8) of batch b for ALL 1024 queries, plus the
partial output projection over its 512 channels. The host sums the two
partial y's per batch during unsharding (a host-side reduce instead of
an on-device collective; assembly was already host-side in v1).

vs v1 ((batch, query-half) sharding, 195.7us): K and V were projected
for the full batch on both cores of a pair (33% redundant FLOPs). Here
every projection is done exactly once: per-core matmul slots drop from
584 to 456 (~29us of TensorE stream), and weight DMA halves (12MB vs
20MB fp32), shrinking the DMA-paced head.

On-chip layout identical to v1 (no transposes anywhere):
  - host passes x^T; K^T/Q^T come out of their projections in [o, t]
  - V in [t, o] with a leading ones-column per head so the PV matmul's
    PSUM row 0 is the softmax denominator Z for free
  - softmax on S^T (keys on partitions) with no max-subtraction
  - normalization: custom-DVE reciprocal of the Z row, GpSimd
    partition_broadcast, one DVE multiply
  - output projection consumes out^T directly, yields y^T (+bias on
    ScalarE, applied only on hh=0 cores; host sums partials)

S^T matmuls contract over head_dim=64 at base partitions 0/64 -> bass
auto-emits them as 64x128 row-tiles T0/T8 which the PE runs
concurrently (verified in the v1 trace: ~4ns start gaps).

Schedule: groups are (pair p, q-half qh), pair-major. q-half-0 groups
are thin (S+PV only); the next pair's K^T/Q^T projections are emitted
inside the q-half-1 groups so every group has >= ~12us of TensorE work
to hide its ~8.8us of ScalarE exp. V streams behind the wv DMA right
after the first S unit; Wproj prefetches ride the idle DMA mid-kernel.
A short keep-warm matmul chain bridges the final norm drain into the
output projection.

All matmuls float32r (TF32-like) with fp32 PSUM accumulation;
KERNEL_BF16=1 switches inputs to bf16 (halves DMA bytes, same matmul
rate, ~4e-3 rel err instead of ~4e-4).
"""

import numpy as np

B, T, C = 4, 1024, 1024
NH, HD = 16, 64
NHL = NH // 2          # heads per core
TQ = T // 2            # q-half tile
KC = C // 128          # 8 contraction chunks over C (x channels)
OC = NHL * HD // 128   # 4 output chunks for K/Q (= local qkv channels/128)
SCALE = 1.0 / float(np.sqrt(HD))

_PROG = None
import os
USE_BF16 = os.environ.get("KERNEL_BF16", "0") == "1"


def _build_program():
    import concourse.bacc as bacc
    import concourse.bass_isa as bass_isa
    import concourse.mybir as mybir
    import concourse.tile as tile

    F32 = mybir.dt.float32
    F32R = mybir.dt.bfloat16 if USE_BF16 else mybir.dt.float32r
    BF16 = mybir.dt.bfloat16
    Exp = mybir.ActivationFunctionType.Exp
    Ident = mybir.ActivationFunctionType.Identity

    def r(ap):
        return ap.bitcast(F32R)

    nc = bacc.Bacc()
    xt_d = nc.declare_dram_parameter("xt", [KC, 128, T], F32R, isOutput=False)
    wq_d = nc.declare_dram_parameter("wq", [OC, 128, C], F32R, isOutput=False)
    wk_d = nc.declare_dram_parameter("wk", [OC, 128, C], F32R, isOutput=False)
    wv_d = nc.declare_dram_parameter("wv", [KC, 128, NHL * 128 // 2], F32R,
                                     isOutput=False)
    wp_d = nc.declare_dram_parameter("wp", [OC, 128, C], F32R, isOutput=False)
    bias_d = nc.declare_dram_parameter("bias", [128, 8], F32, isOutput=False)
    ones_d = nc.declare_dram_parameter("ones", [128, 128], F32R, isOutput=False)
    yt_d = nc.declare_dram_parameter("yt", [8, 128, T], F32, isOutput=True)

    from contextlib import ExitStack

    with ExitStack() as ctx:
        tc = ctx.enter_context(tile.TileContext(nc))
        ctx.enter_context(
            nc.allow_low_precision(
                "float32r matmul inputs (TF32-like) are intentional"
            )
        )
        pool = lambda name, bufs, **kw: ctx.enter_context(  # noqa: E731
            tc.tile_pool(name=name, bufs=bufs, **kw)
        )
        xt_pool = pool("xt", KC)
        wstat_pool = pool("wstat", 3)
        wv_pool = pool("wv", KC)
        wp_pool = pool("wp", OC)
        kt_pool = pool("kt", 2)
        qt_pool = pool("qt", 2)
        v_pool = pool("vaug", KC)
        exp_pool = pool("exp", 10)
        ot_pool = pool("ot", OC)
        y_pool = pool("ysb", 4)
        osb2_pool = pool("osb2", 2)
        r_pool = pool("rsb", 2)
        rbx_pool = pool("rbx", 2)
        bias_pool = pool("bias", 8)
        ps_proj = pool("psproj", 2, space="PSUM")
        ps_s = pool("pss", 2, space="PSUM")  # [128,1024] pair tiles, 2 banks
        ps_ops = pool("psops", 2, space="PSUM")

        # ---- stage 0: streamed loads ----------------------------------
        # The first-half weights go out on the ScalarE HWDGE ring (its
        # preamble finishes earlier than Sync's and the rings dispatch
        # in parallel); xt and everything else stream on Sync. Only
        # 0.83MB precedes xt0, and the n=1 weight halves (contraction
        # chunks 4-7, not needed until xt4) follow xt0.
        ones_sb = bias_pool.tile([128, 128], F32R, tag="ones", name="ones_sb")
        nc.scalar.dma_start(ones_sb[:], ones_d[:])

        def rf(ap):
            # view an F32R-declared tile as plain fp32 for DVE arithmetic
            return ap if USE_BF16 else ap.bitcast(F32)
        kw0 = wstat_pool.tile([128, C], F32R, tag="wstat", name="kw0")
        nc.scalar.dma_start(kw0[:, 0:TQ], wk_d[0][:, 0:TQ])
        qw0 = wstat_pool.tile([128, C], F32R, tag="wstat", name="qw0")
        nc.scalar.dma_start(qw0[:, 0:TQ], wq_d[0][:, 0:TQ])
        xt = []
        for k in range(KC):
            t_ = xt_pool.tile([128, T], F32R, tag="xt", name=f"xt{k}")
            nc.sync.dma_start(t_[:], xt_d[k])
            xt.append(t_)
            if k == 0:
                nc.sync.dma_start(kw0[:, TQ:C], wk_d[0][:, TQ:C])
                nc.sync.dma_start(qw0[:, TQ:C], wq_d[0][:, TQ:C])

        VW = NHL * 64  # 512 V output cols
        wv_sb = []
        for k in range(KC):
            wvt = wv_pool.tile([128, VW], F32R, tag="wv", name=f"wv{k}")
            nc.sync.dma_start(wvt[:], wv_d[k])
            wv_sb.append(wvt)
        va = []
        for m in range(KC):
            vt = v_pool.tile([128, NHL * 65], F32R, tag="vaug", name=f"va{m}")
            view = vt[:].rearrange("p (h e) -> p h e", e=65)
            # ones column FIRST per head: the PV output's Z row lands on
            # partition 0 (softmax denominator for free).
            nc.vector.tensor_copy(view[:, :, 0:1], ones_sb[:, 0:NHL].unsqueeze(2))
            va.append(vt)

        bias_t = bias_pool.tile([128, 8], F32, tag="bias", name="bias_t")
        nc.sync.dma_start(bias_t[:], bias_d[:])
        bias_sb = [bias_t[:, m:m + 1] for m in range(8)]

        ot = []
        for k in range(OC):
            o_ = ot_pool.tile([128, T], F32R, tag="ot", name=f"ot{k}")
            ot.append(o_)

        def emit_proj_half(t_, w, p, n, which):
            # one T/2 half of a K^T or Q^T chunk: [128 (2 heads x 64d), TQ]
            ps = ps_proj.tile([128, TQ], F32, tag="ps", name=f"{which}{p}{n}")
            for k in range(KC):
                nc.tensor.matmul(
                    ps[:], r(w[:, k * 128:(k + 1) * 128]),
                    r(xt[k][:, n * TQ:(n + 1) * TQ]),
                    start=(k == 0), stop=(k == KC - 1),
                )
            nc.vector.tensor_copy(t_[:, n * TQ:(n + 1) * TQ], ps[:])

        def emit_proj(p, w, which):
            t_ = (kt_pool if which == "k" else qt_pool).tile(
                [128, T], F32R,
                tag="kt" if which == "k" else "qt", name=f"{which}t{p}")
            for n in range(2):
                emit_proj_half(t_, w, p, n, which)
            return t_

        def emit_s_pair(p, qh, k_, q_):
            # Both heads' S^T chunk j share one 2-bank PSUM tile so a
            # single [128,1024] exp covers them. The two matmuls are
            # 64x128 row-tiles (base partitions 0/64) -> concurrent.
            exps = []
            for j in range(KC):
                sps = ps_s.tile([128, 2 * TQ], F32, tag="ps", name=f"s{p}{qh}{j}")
                nc.tensor.matmul(
                    sps[:, 0:TQ],
                    r(k_[0:64, j * 128:(j + 1) * 128]),
                    r(q_[0:64, qh * TQ:(qh + 1) * TQ]),
                    start=True, stop=True,
                )
                nc.tensor.matmul(
                    sps[:, TQ:2 * TQ],
                    r(k_[64:128, j * 128:(j + 1) * 128]),
                    r(q_[64:128, qh * TQ:(qh + 1) * TQ]),
                    start=True, stop=True,
                )
                e = exp_pool.tile([128, 2 * TQ], F32R, tag="exp",
                                  name=f"e{p}{qh}{j}")
                nc.scalar.activation(e[:], sps[:], Exp, scale=SCALE)
                exps.append(e)
            return exps

        def emit_norm(hl, qh, ops, fast=False):
            okc, half = divmod(hl, 2)
            po = half * 64
            if fast:
                # Last-unit path: minimum-latency chain straight from
                # PSUM (nothing waits on the ps_ops buf anymore).
                src = ops[0:65, :]
            else:
                # Evacuate PSUM first: a single DVE copy frees the
                # ps_ops buf for the next PV ~2.3us earlier than holding
                # it through the whole recip/broadcast/mul chain (which
                # paced PV starts in the 2-buf rotation).
                src = osb2_pool.tile([65, TQ], F32, tag="oraw",
                                     name=f"or{hl}{qh}")[:]
                nc.vector.tensor_copy(src, ops[0:65, :])
            rt0 = r_pool.tile([1, TQ], F32, tag="rsb", name=f"r0{hl}{qh}")
            nc.vector.reciprocal_approx_fast(rt0[0:1, :], src[0:1, :])
            rbx = rbx_pool.tile([65, TQ], F32, tag="rbx", name=f"rbx{hl}{qh}")
            nc.gpsimd.partition_broadcast(rbx[:], rt0[0:1, :])
            ob2 = osb2_pool.tile([65, TQ], F32R, tag="osb2", name=f"ob2_{hl}{qh}")
            nc.vector.tensor_mul(ob2[:], src, rbx[:])
            nc.sync.dma_start(ot[okc][po:po + 64, qh * TQ:(qh + 1) * TQ],
                              ob2[1:65, :])

        def emit_pv(hl, qh, exps, fast_norm=False):
            lo = (hl % 2) * TQ
            ops = ps_ops.tile([65, TQ], F32, tag="ps", name=f"o{hl}{qh}")
            for j in range(KC):
                nc.tensor.matmul(
                    ops[:], r(va[j][:, hl * 65:(hl + 1) * 65]),
                    r(exps[j][:, lo:lo + TQ]),
                    start=(j == 0), stop=(j == KC - 1),
                )
            emit_norm(hl, qh, ops, fast=fast_norm)

        # ---- pair 0 header: 4-way interleaved chase -------------------
        # All four kt0/qt0 half-accumulators advance together as each xt
        # chunk's DMA lands (4 matmuls per arrival instead of serial
        # half-by-half chasing), with short ones x ones filler matmuls
        # between chunks to keep the PE's HAM activity window warm while
        # the stream is DMA-paced.
        kt = kt_pool.tile([128, T], F32R, tag="kt", name="kt0")
        qt = qt_pool.tile([128, T], F32R, tag="qt", name="qt0")
        ps_k0 = ps_proj.tile([128, TQ], F32, tag="ps", name="ps_k0")
        ps_k1 = ps_proj.tile([128, TQ], F32, tag="ps", name="ps_k1")
        ps_q = ps_s.tile([128, 2 * TQ], F32, tag="ps", name="ps_q01")
        warm0 = ps_ops.tile([128, TQ], F32, tag="ps", name="warm0")
        for k in range(KC):
            fl = dict(start=(k == 0), stop=(k == KC - 1))
            nc.tensor.matmul(ps_k0[:], r(kw0[:, k * 128:(k + 1) * 128]),
                             r(xt[k][:, 0:TQ]), **fl)
            nc.tensor.matmul(ps_k1[:], r(kw0[:, k * 128:(k + 1) * 128]),
                             r(xt[k][:, TQ:T]), **fl)
            nc.tensor.matmul(ps_q[:, 0:TQ], r(qw0[:, k * 128:(k + 1) * 128]),
                             r(xt[k][:, 0:TQ]), **fl)
            nc.tensor.matmul(ps_q[:, TQ:T], r(qw0[:, k * 128:(k + 1) * 128]),
                             r(xt[k][:, TQ:T]), **fl)
            if k < KC - 1:
                # fillers read xt[k] so the scheduler can't hoist them
                # ahead of the chase — they run in the DMA-wait window
                # after chunk k's projections. Dense enough (~0.45us per
                # ~1.4us chunk gap) to push the HAM activity window into
                # K=8/8 mid-chase instead of ~21us in.
                for w in range(2):
                    nc.tensor.matmul(warm0[:, 0:TQ], r(ones_sb[:]),
                                     r(xt[k][:, 0:TQ]), start=True, stop=True)
        nc.vector.tensor_copy(kt[:, 0:TQ], ps_k0[:])
        nc.vector.tensor_copy(kt[:, TQ:T], ps_k1[:])
        nc.vector.tensor_copy(qt[:, 0:TQ], ps_q[:, 0:TQ])
        nc.vector.tensor_copy(qt[:, TQ:T], ps_q[:, TQ:T])
        exps = emit_s_pair(0, 0, kt, qt)

        # ---- V = x @ Wv^T ([t,o] + ones cols), streams behind DMA -----
        for m in range(KC):
            view = va[m][:].rearrange("p (h e) -> p h e", e=65)
            ps = ps_proj.tile([128, VW], F32, tag="ps", name=f"v{m}")
            for k in range(KC):
                nc.tensor.matmul(
                    ps[:], r(xt[k][:, m * 128:(m + 1) * 128]),
                    r(wv_sb[k][:]),
                    start=(k == 0), stop=(k == KC - 1),
                )
            src = ps[:].rearrange("p (h d) -> p h d", d=64)
            nc.vector.tensor_copy(view[:, 0:NHL, 1:65], src)

        # ---- groups, pair-major --------------------------------------
        # Each group of pair p also carries HALF of pair p+1's K^T/Q^T
        # projections (16 matmul slots) so every group has ~9us of
        # TensorE work against its ~8.8us of ScalarE exp; Wproj
        # prefetches ride the idle DMA.
        wp_sb = {}

        def prefetch_wp(m):
            yw = wp_pool.tile([128, C], F32R, tag="wp", name=f"yw{m}")
            nc.sync.dma_start(yw[:], wp_d[m])
            wp_sb[m] = yw

        # y^T partial = Wproj_half @ out^T (+ b on hh=0). wp_sb[k]
        # covers contraction chunk k (128 of the core's 512 channels)
        # with all 1024 outputs in its cols; output chunk m uses cols
        # m*128:(m+1)*128. Blocks rotate across ALL three PSUM pools
        # (S/PV pools are dead by then) so the bias-ACT evacuation never
        # gates the matmul stream. th=0 blocks only need the q-half-0
        # norms, so they run inside the last group (filling its exp
        # window); th=1 blocks are the kernel tail.
        y_ps_pools = [ps_proj, ps_s, ps_ops]

        def emit_y_half(th):
            for m in range(8):
                ps = y_ps_pools[m % 3].tile([128, TQ], F32, tag="ps",
                                            name=f"y{m}{th}")
                for k in range(OC):
                    nc.tensor.matmul(
                        ps[:], r(wp_sb[k][:, m * 128:(m + 1) * 128]),
                        r(ot[k][:, th * TQ:(th + 1) * TQ]),
                        start=(k == 0), stop=(k == OC - 1),
                    )
                ysb = y_pool.tile([128, TQ], F32, tag="ysb", name=f"ysb{m}{th}")
                nc.scalar.activation(ysb[:], ps[:], Ident, bias=bias_sb[m])
                nc.sync.dma_start(yt_d[m][:, th * TQ:(th + 1) * TQ], ysb[:])

        kt_nx = qt_nx = kw_nx = qw_nx = None
        for g in range(2 * OC):
            p, qh = divmod(g, 2)
            if g > 0:
                exps = emit_s_pair(p, qh, kt, qt)
            if g == 2 * OC - 1:
                prefetch_wp(OC - 1)
            if qh == 0:
                if p + 1 < OC:
                    kw_nx = wstat_pool.tile([128, C], F32R, tag="wstat",
                                            name=f"kw{p + 1}")
                    nc.sync.dma_start(kw_nx[:], wk_d[p + 1])
                    qw_nx = wstat_pool.tile([128, C], F32R, tag="wstat",
                                            name=f"qw{p + 1}")
                    nc.sync.dma_start(qw_nx[:], wq_d[p + 1])
                    kt_nx = kt_pool.tile([128, T], F32R, tag="kt",
                                         name=f"kt{p + 1}")
                    qt_nx = qt_pool.tile([128, T], F32R, tag="qt",
                                         name=f"qt{p + 1}")
                    emit_proj_half(kt_nx, kw_nx, p + 1, 0, "k")
                    emit_proj_half(qt_nx, qw_nx, p + 1, 0, "q")
            else:
                if p + 1 < OC:
                    emit_proj_half(kt_nx, kw_nx, p + 1, 1, "k")
                    emit_proj_half(qt_nx, qw_nx, p + 1, 1, "q")
            if g == 2 * OC - 1:
                emit_y_half(0)
            last = g == 2 * OC - 1
            emit_pv(2 * p, qh, exps, fast_norm=last)
            emit_pv(2 * p + 1, qh, exps, fast_norm=last)
            if qh == 1:
                if p + 1 < OC:
                    kt, qt = kt_nx, qt_nx
                for m in {1: [0], 2: [1, 2]}.get(p, []):
                    prefetch_wp(m)

        # ---- Y th=1 tail: incremental accumulation --------------------
        # m=0..5 pre-accumulate contraction chunks k=0..2 (their ot
        # chunks were normed a group ago) in ps_proj + ps_s banks while
        # the final norm chains drain (doubles as the HAM keep-warm
        # bridge); only their k=3 stop-matmuls and the m=6,7 blocks
        # (ps_ops, free after the final norm muls) sit past the last
        # norm.
        y_aps = []
        for i in range(2):
            t = ps_proj.tile([128, TQ], F32, tag="ps", name=f"yt1a{i}")
            y_aps.append(t[:])
        for i in range(2):
            t = ps_s.tile([128, 2 * TQ], F32, tag="ps", name=f"yt1c{i}")
            y_aps.append(t[:, 0:TQ])
            y_aps.append(t[:, TQ:2 * TQ])
        for k in range(OC - 1):
            for m in range(6):
                nc.tensor.matmul(
                    y_aps[m], r(wp_sb[k][:, m * 128:(m + 1) * 128]),
                    r(ot[k][:, TQ:T]),
                    start=(k == 0), stop=False,
                    skip_group_check=True,
                )
        for m in range(6):
            nc.tensor.matmul(
                y_aps[m], r(wp_sb[OC - 1][:, m * 128:(m + 1) * 128]),
                r(ot[OC - 1][:, TQ:T]),
                start=False, stop=True,
                skip_group_check=True,
            )
            ysb = y_pool.tile([128, TQ], F32, tag="ysb", name=f"ysb{m}1")
            nc.scalar.activation(ysb[:], y_aps[m], Ident, bias=bias_sb[m])
            nc.sync.dma_start(yt_d[m][:, TQ:T], ysb[:])
        for m in (6, 7):
            ps = ps_ops.tile([128, TQ], F32, tag="ps", name=f"y{m}1")
            for k in range(OC):
                nc.tensor.matmul(
                    ps[:], r(wp_sb[k][:, m * 128:(m + 1) * 128]),
                    r(ot[k][:, TQ:T]),
                    start=(k == 0), stop=(k == OC - 1),
                )
            ysb = y_pool.tile([128, TQ], F32, tag="ysb", name=f"ysb{m}1")
            nc.scalar.activation(ysb[:], ps[:], Ident, bias=bias_sb[m])
            nc.sync.dma_start(yt_d[m][:, TQ:T], ysb[:])

    nc.compile()
    return nc


def _get_program():
    global _PROG
    if _PROG is None:
        _PROG = _build_program()
    return _PROG


def _prep_inputs(x, Wqkv, Wproj, bproj):
    """Host-side shard prep: per-core input maps (contiguous)."""
    x = np.asarray(x, dtype=np.float32)
    Wqkv = np.asarray(Wqkv, dtype=np.float32)
    Wproj = np.asarray(Wproj, dtype=np.float32)
    bproj = np.asarray(bproj, dtype=np.float32)

    mmdt = np.float32
    if USE_BF16:
        import ml_dtypes

        mmdt = ml_dtypes.bfloat16

    def cols(wT):
        # [C, 512] (c, o) -> [OC, 128, C]: per o-chunk column, laid out
        # so one contiguous DMA fills the stationary tile
        # [128p, k*128+d] = wT[k*128+p, oc*128+d]
        return np.ascontiguousarray(
            wT.reshape(KC, 128, OC, 128).transpose(2, 1, 0, 3).reshape(OC, 128, C)
        )

    halves = []
    for hh in range(2):
        sl = slice(hh * 512, (hh + 1) * 512)
        wq = cols(Wqkv[0:C][sl].T).astype(mmdt)
        wk = cols(Wqkv[C:2 * C][sl].T).astype(mmdt)
        wv = np.ascontiguousarray(
            Wqkv[2 * C:3 * C][sl].T.reshape(KC, 128, 512)
        ).astype(mmdt)
        # wp chunk k: [128 c, 1024 o] = Wproj[:, hh*512 + k*128 + p].T
        wp = np.ascontiguousarray(
            Wproj[:, sl].T.reshape(OC, 128, C)
        ).astype(mmdt)
        bias = np.ascontiguousarray(
            (bproj if hh == 0 else np.zeros_like(bproj)).reshape(8, 128).T
        )
        halves.append((wq, wk, wv, wp, bias))
    ones = np.ones((128, 128), dtype=mmdt)

    in_maps = []
    for i in range(8):
        b, hh = divmod(i, 2)
        wq, wk, wv, wp, bias = halves[hh]
        xt = np.ascontiguousarray(x[b].T.reshape(KC, 128, T)).astype(mmdt)
        in_maps.append(
            {
                "xt": xt, "wq": wq, "wk": wk, "wv": wv, "wp": wp,
                "bias": bias, "ones": ones,
            }
        )
    return in_maps


def _assemble(results, x_dtype):
    out = np.empty((B, T, C), dtype=np.float32)
    for b in range(B):
        y0 = results[2 * b]["yt"].reshape(C, T)
        y1 = results[2 * b + 1]["yt"].reshape(C, T)
        out[b] = (y0 + y1).T
    return out.astype(x_dtype, copy=False)


def run(inputs, trace=False, **spmd_kwargs):
    """Shared entry for kernel() and test harnesses (trace for profiling)."""
    from concourse.bass_utils import run_bass_kernel_spmd

    nc = _get_program()
    in_maps = _prep_inputs(**inputs)
    res = run_bass_kernel_spmd(
        nc, in_maps, list(range(8)), trace=trace, **spmd_kwargs
    )
    out = _assemble(res.results, np.asarray(inputs["x"]).dtype)
    return out, res


def kernel(x, Wqkv, Wproj, bproj):
    out, _ = run(dict(x=x, Wqkv=Wqkv, Wproj=Wproj, bproj=bproj))
    return out
